# revision 20
# baseline (speedup 1.0000x reference)
"""CTC loss (keras ctc_batch_cost semantics) on 8 Trainium2 NeuronCores.

Data parallel: 32 examples per core. The sequential alpha recurrence runs in
the probability domain with periodic rescaling (every 32 steps):

    alpha_t = q_t * (A_b @ alpha_{t-1}),   q_t[s] = 512*(y_pred[b,t,ext[s]]+EPS)

with states on partitions ([97, batch] layout). The per-example banded
transition matrix A_b = (I+S1) + S2*diag(mask) is applied by the tensor engine
as two PSUM-accumulating matmuls with shared 0/1 weights: the skip mask is
folded into a second coefficient stream r_t = mask_shift2 * q_t, so

    z_t = W1 @ (q_t*z_{t-1}) + W2 @ (r_t*z_{t-1})

and the vector engine does ONE fused multiply per step producing
[u|v] = [q_t|r_t] * dup(z_{t-1}), reading z straight from PSUM.

loss = -(log(u_T[95]+u_T[96]) + sum_j log(c_j) - T*log(512)).

The (t,s) coefficient gather (a label-indexed take on y_pred) is precomputed
on the host and uploaded as a compact bf16 tensor [97, T, 2, 32] per core
(6.4 MB vs 134 MB for raw y_pred); the recurrence accumulates in f32 on
device. End-to-end numpy emulation matches the jax reference to ~2e-4 max rel
err (bf16 coefficient rounding; f32 variant matches to 2e-6).

NOTE on DMA structure: this walrus build lowers DMA/memset to pseudo-DMA
instructions that accept at most ONE sync-wait command, so the program keeps
all loads write-once/dependency-free and budgets < 8 DMA-lowered instructions
before the single (dependency-carrying) loss store.
"""
import os
import sys
import numpy as np

for _p in ("/opt/trn_rl_repo", "/root/.axon_site/_ro/trn_rl_repo"):
    if os.path.isdir(_p) and _p not in sys.path:
        sys.path.insert(0, _p)

import ml_dtypes  # noqa: E402
import concourse.bass as bass  # noqa: E402
import concourse.bacc as bacc  # noqa: E402
import concourse.mybir as mybir  # noqa: E402
import concourse.tile as tile  # noqa: E402
from concourse.bass_utils import run_bass_kernel_spmd  # noqa: E402

BF = ml_dtypes.bfloat16
F32 = np.float32

B, T, L, C = 256, 512, 48, 512
S = 2 * L + 1          # 97
BLANK = C - 1
EPS = 1e-7
ZQ = 512.0             # per-step scale folded into the coefficients
NCORES = 8
BPC = B // NCORES      # 32 examples per core
RESC = 32              # rescale interval
NCHUNK = 3             # qr load chunks (t-sliced; chunk 0 carries e01 slot)


def _resc_ts(Tt):
    return [t for t in range(RESC, Tt - 1, RESC)]


def _chunk_bounds(Tt):
    per = -(-Tt // NCHUNK)
    bounds = []
    lo = 0
    for _ in range(NCHUNK):
        hi = min(lo + per, Tt)
        bounds.append((lo, hi))
        lo = hi
    return bounds


# ---------------------------------------------------------------------------
# host-side precompute
# ---------------------------------------------------------------------------

def host_coeffs(y_true, y_pred):
    """QR host tensor [S, Tt+1, 2, n] bf16 (slot Tt = e01 init).

    q[s,t,b] = ZQ*(y_pred[b,t,ext[b,s]] + EPS);  r = mask_shift2 * q.
    """
    lab = np.asarray(y_true).astype(np.int64)
    y = np.asarray(y_pred, dtype=F32)
    n, Tt = lab.shape[0], y.shape[1]

    ext = np.full((n, S), BLANK, dtype=np.int64)
    ext[:, 1::2] = lab
    m = np.zeros((n, S), dtype=F32)
    m[:, 1] = 1.0
    odd = np.arange(3, S, 2)
    m[:, odd] = (ext[:, odd] != ext[:, odd - 2]).astype(F32)
    md2 = np.zeros((n, S), dtype=F32)
    md2[:, :S - 2] = m[:, 2:]

    q = np.take_along_axis(y, ext[:, None, :], axis=2) + EPS  # [n, Tt, S]
    q *= ZQ
    r = q * md2[:, None, :]
    qr = np.stack([q, r], axis=2)            # [n, Tt, 2, S]
    qr = qr.transpose(3, 1, 2, 0)            # [S, Tt, 2, n]
    H = np.zeros((S, Tt + 1, 2, n), dtype=F32)
    H[:, :Tt] = qr
    H[0:2, Tt, 0, :] = 1.0                   # e01 init (k=0 slice)
    return H.astype(BF)


def host_aux():
    """aux [S, 196] bf16: W1 | W2 | ones-col | sel-col."""
    aux = np.zeros((S, 196), dtype=F32)
    ss = np.arange(S)
    aux[ss, ss] = 1.0                        # W1: k == s
    aux[ss[1:] - 1, ss[1:]] = 1.0            # W1: k == s-1
    aux[ss[2:] - 2, 97 + ss[2:]] = 1.0       # W2: k == s-2
    aux[:, 194] = 1.0                        # ones column (csum)
    aux[95:97, 195] = 1.0                    # final-state selector
    return aux.astype(BF)


# ---------------------------------------------------------------------------
# device program
# ---------------------------------------------------------------------------

def build_bass(n_ex=BPC, Tt=T, debug=False):
    dtb = mybir.dt.bfloat16
    dtf = mybir.dt.float32
    resc_ts = _resc_ts(Tt)
    ncs = len(resc_ts) + 1
    bounds = _chunk_bounds(Tt)

    nc = bacc.Bacc()
    qr_d = nc.dram_tensor("qr", [S, Tt + 1, 2, n_ex], dtb,
                          kind="ExternalInput")
    aux_d = nc.dram_tensor("aux", [S, 196], dtb, kind="ExternalInput")
    loss_d = nc.dram_tensor("loss", [n_ex, 1], dtf, kind="ExternalOutput")

    with tile.TileContext(nc) as tc:
        with (
            tc.tile_pool(name="persist", bufs=1) as persist,
            tc.tile_pool(name="uv", bufs=2) as uv_pool,
            tc.tile_pool(name="zp", bufs=2, space="PSUM") as zP,
            tc.tile_pool(name="csp", bufs=1, space="PSUM") as csP,
        ):
            qr_t = []
            for ci, (lo, hi) in enumerate(bounds):
                slots = hi - lo + (1 if ci == 0 else 0)
                qt = persist.tile([S, slots, 2, n_ex], dtb, tag=f"qr{ci}")
                qr_t.append(qt)
            aux_t = persist.tile([S, 196], dtb, tag="aux")
            cbuf = persist.tile([1, ncs, n_ex], dtf, tag="cbuf")
            logbuf = persist.tile([1, ncs, n_ex], dtf, tag="logbuf")
            rscale = persist.tile([1, n_ex], dtf, tag="rscale")
            rb_s = persist.tile([S, n_ex], dtf, tag="rb_s")
            llsum = persist.tile([1, n_ex], dtf, tag="llsum")
            lossb = persist.tile([1, n_ex], dtf, tag="lossb")

            (lo0, hi0) = bounds[0]
            nc.gpsimd.dma_start(qr_t[0][:, 0:hi0 - lo0, :, :],
                                qr_d[:, lo0:hi0, :, :])
            nc.gpsimd.dma_start(qr_t[0][:, hi0 - lo0, :, :],
                                qr_d[:, Tt, :, :])
            for ci in range(1, NCHUNK):
                lo, hi = bounds[ci]
                nc.gpsimd.dma_start(qr_t[ci][:], qr_d[:, lo:hi, :, :])
            nc.gpsimd.dma_start(aux_t[:], aux_d[:])

            w1 = aux_t[:, 0:97]
            w2 = aux_t[:, 97:194]
            ones_col = aux_t[:, 194:195]
            sel_col = aux_t[:, 195:196]

            def qr_slot(t):
                for ci, (lo, hi) in enumerate(bounds):
                    if lo <= t < hi:
                        return qr_t[ci][:, t - lo, :, :]
                raise AssertionError(t)

            e01_slot = qr_t[0][:, hi0 - lo0, :, :]   # [S, 2, n_ex]

            # ---------------- chain ----------------
            uvA = uv_pool.tile([S, 2, n_ex], dtb, tag="uvA")
            uvB = uv_pool.tile([S, 2, n_ex], dtb, tag="uvB")

            z_prev = None
            uv = None
            for t in range(Tt):
                uv = uvA if (t % 2 == 0) else uvB
                if t == 0:
                    src = e01_slot[:, 0, :].unsqueeze(1).broadcast_to(
                        [S, 2, n_ex])
                else:
                    src = z_prev[:].unsqueeze(1).broadcast_to([S, 2, n_ex])
                nc.vector.tensor_tensor(uv[:], src, qr_slot(t),
                                        mybir.AluOpType.mult)
                if t in resc_ts:
                    j = resc_ts.index(t)
                    cs = csP.tile([1, n_ex], dtf, tag="cs")
                    nc.tensor.matmul(cs[:], ones_col, uv[:, 0, :],
                                     start=True, stop=True)
                    nc.vector.reciprocal(rscale[:], cs[:])
                    nc.scalar.copy(cbuf[:, j, :], cs[:])
                    nc.gpsimd.partition_broadcast(rb_s[:], rscale[:])
                    rbb = rb_s[:].unsqueeze(1).broadcast_to([S, 2, n_ex])
                    nc.vector.tensor_tensor(uv[:], uv[:], rbb,
                                            mybir.AluOpType.mult)
                if t < Tt - 1:
                    z = zP.tile([S, n_ex], dtf, tag="z")
                    nc.tensor.matmul(z[:], w1, uv[:, 0, :],
                                     start=True, stop=False)
                    nc.tensor.matmul(z[:], w2, uv[:, 1, :],
                                     start=False, stop=True)
                    z_prev = z

            # ---------------- finalize ----------------
            fin = csP.tile([1, n_ex], dtf, tag="cs")
            nc.tensor.matmul(fin[:], sel_col, uv[:, 0, :],
                             start=True, stop=True)
            nc.scalar.copy(cbuf[:, ncs - 1, :], fin[:])
            nc.scalar.activation(logbuf[:], cbuf[:],
                                 mybir.ActivationFunctionType.Ln)
            nc.vector.tensor_reduce(
                llsum[:], logbuf[:].rearrange("p j b -> p b j"),
                mybir.AxisListType.X, mybir.AluOpType.add)
            # loss = -llsum + T*log(ZQ), computed on ACT. Written twice:
            # the store DMA's wait fires on engine-sem completion, which can
            # race the SBUF write's visibility on the DMA port (observed on
            # HW: stale bytes beyond element 0). The second identical write
            # makes any race expose the first write's identical data.
            for _ in range(2):
                nc.scalar.activation(lossb[:], llsum[:],
                                     mybir.ActivationFunctionType.Copy,
                                     bias=float(Tt * np.log(ZQ)), scale=-1.0)
            nc.gpsimd.dma_start(loss_d[:, 0].unsqueeze(0), lossb[0:1, :])
    nc.compile()
    return nc


# ---------------------------------------------------------------------------
# entry point
# ---------------------------------------------------------------------------

_CACHE = {}


def _get_nc():
    if "nc" not in _CACHE:
        _CACHE["nc"] = build_bass()
    return _CACHE["nc"]


def make_in_maps(y_true, y_pred):
    y_true = np.asarray(y_true)
    y_pred = np.asarray(y_pred, dtype=F32)
    aux = host_aux()
    in_maps = []
    for core in range(NCORES):
        sl = slice(core * BPC, (core + 1) * BPC)
        in_maps.append({
            "qr": host_coeffs(y_true[sl], y_pred[sl]),
            "aux": aux,
        })
    return in_maps


def kernel(y_true, y_pred):
    nc = _get_nc()
    in_maps = make_in_maps(y_true, y_pred)
    res = run_bass_kernel_spmd(nc, in_maps, list(range(NCORES)))
    out = np.concatenate([res.results[c]["loss"] for c in range(NCORES)],
                         axis=0)
    return out.astype(F32)
